# revision 10
# baseline (speedup 1.0000x reference)
"""Trainium2 Bass kernel for nn_LlamaApproximatedAttention.

Math (per batch b, with hs = hidden_states[b] [S, H]):
    F_h = W_seq @ hs            # [R, H]   (contract s)
    F_s = hs @ W_hid.T          # [S, R]   (contract h)
    out = F_s @ F_h             # [S, H]   (contract r)

Sharding: 8 cores = (batch b = c//2, seq-half j = c%2). Each core receives the
full hs[b] with its own half's rows first (host-side roll), computes F_h fully,
and F_s / out only for its own half. Pure SPMD.

DMA layout: hs is pre-packed on the host into 4 groups of 4 s-tiles,
[g][p][tile*H] fp16, so each group load is a single 16KB-contiguous descriptor
per partition (128 descriptors total). The output uses the same packing
([ip][p][w*H], 8KB/partition/store). Loads go on the SP DGE queue, stores on
the Pool (SWDGE) queue so stores never block next-iteration load triggers.

SBUF pools are opened OUTSIDE the timing loop and the loop body is emitted
twice per hardware For iteration, so tile-pool buffer rotation carries across
iterations (cross-iteration load prefetch / store draining). Constants load
once, outside the loop.

All PE inputs are float16 (PSUM accumulation stays fp32): halves DMA traffic
(the bottleneck) vs f32, and PE transposes run 1.0 cycles/row vs 1.5 for f32r.
grid_chw is unused by the math (it enumerates the full (s, h) grid).
"""

import numpy as np

import concourse.bass as bass  # noqa: F401  (engine namespaces hang off nc)
import concourse.mybir as mybir
import concourse.tile as tile
from concourse import bacc
from concourse.bass_utils import run_bass_kernel_spmd

B, S, H, R = 4, 2048, 2048, 64
N_CORES = 8
P = 128
T = S // P            # 16 s-tiles (also 16 h-tiles)
OWN_T = T // 2        # 8 own s-tiles per core
CHUNK = 512
NCH = H // CHUNK      # 4 h-chunks
NG = 4                # s-tile groups of 4 (load granularity)

f16 = mybir.dt.float16
f32 = mybir.dt.float32

# Pair-wise AllReduce of the partial F_h: each core loads only its own seq
# half of hs (halves load DMA + einsum1) and the (b, j=0)/(b, j=1) pair
# reduces F_h over a device-to-device collective.
COLLECTIVE = False
CC_GROUPS = [[0, 1], [2, 3], [4, 5], [6, 7]]


def build_nc(reps: int = 1, mode: str = "full", unroll: bool = False):
    """Build + bacc-compile the SPMD kernel. reps>1 wraps two copies of the
    body in a hardware For loop of reps//2 (reps must be even), so tile-pool
    rotation double-buffers across iterations. Output is idempotent.

    mode (timing diagnostics, progressively enables phases):
      "dma"  : input loads + output stores only
      "e1"   : + einsum1 matmuls
      "tr"   : + PE transposes
      "e2"   : + einsum2 matmuls/copies
      "full" : + einsum3 (the real kernel)
    mode may carry suffixes "-noload" (skip input DMAs; compute on garbage
    SBUF) and/or "-nostore" (skip output DMAs) for bottleneck isolation.
    """
    skip_loads = "-noload" in mode
    skip_stores = "-nostore" in mode
    mode = mode.split("-")[0]
    lvl = {"dma": 0, "e1": 1, "tr": 2, "e2": 3, "full": 4}[mode]
    nc = bacc.Bacc(
        "TRN2",
        target_bir_lowering=False,
        debug=False,
        enable_asserts=True,
        num_devices=N_CORES,
    )

    # group g holds s-tiles 4g..4g+3 (of the rows this core receives);
    # per-partition line is 4*H fp16 = 16KB contiguous
    ngl = 2 if COLLECTIVE else NG       # groups of 4 s-tiles used by einsum1
    n_t = 4 * ngl                       # s-tiles contracted by einsum1 here
    hsg = nc.dram_tensor("hsg", [ngl, P, 4 * H], f16, kind="ExternalInput").ap()
    wst = nc.dram_tensor("wst", [P, n_t * R], f16, kind="ExternalInput").ap()
    wht = nc.dram_tensor("wht", [P, T * R], f16, kind="ExternalInput").ap()
    ident = nc.dram_tensor("ident", [P, P], f16, kind="ExternalInput").ap()
    # store pair sp holds s-tiles 4sp..4sp+3; 4*H fp16 = 16KB/partition
    out = nc.dram_tensor("out", [NG // 2, P, 4 * H], f16, kind="ExternalOutput").ap()

    with tile.TileContext(nc) as tc:
        with (
            tc.tile_pool(name="const", bufs=1) as cpool,
            tc.tile_pool(name="raw", bufs=2) as rpool,
            tc.tile_pool(name="hsT", bufs=8) as hpool,
            tc.tile_pool(name="fact", bufs=2) as fpool,
            tc.tile_pool(name="outsb", bufs=2) as opool,
            tc.tile_pool(name="ccb", bufs=2, space="DRAM") as ccpool,
        ):
            # constants: loaded once, outside the timing loop
            wst_t = cpool.tile([P, n_t * R], f16, tag="wst")
            wht_t = cpool.tile([P, T * R], f16, tag="wht")
            id_t = cpool.tile([P, P], f16, tag="id")
            nc.gpsimd.dma_start(wst_t[:], wst)
            nc.gpsimd.dma_start(wht_t[:], wht)
            nc.gpsimd.dma_start(id_t[:], ident)
            fake_out = None
            if lvl < 4:
                fake_out = cpool.tile([P, 4 * H], f16, tag="fake_out")
                nc.gpsimd.memset(fake_out[:], 0.0)

            def body(_=None):
                # per-group F_s.T tiles so einsum3's ip0/ip1 depend only on
                # group 0's copies (a single tile would serialize on group 1)
                fst_g = [
                    fpool.tile([P, 4 * P], f16, tag=f"fst{g}", name=f"fst{g}")
                    for g in range(2)
                ]
                fh_dup = fpool.tile([P, H], f16, tag="fh")            # [128, 2048]

                # one 16KB/partition load per group; raws[g] is the
                # (tile, column offset) of group g within its load tile
                raws = []
                for g in range(ngl):
                    raw = rpool.tile([P, 4 * H], f16, tag=f"raw{g}")
                    if not skip_loads:
                        nc.sync.dma_start(raw[:], hsg[g])
                    raws.append((raw, 0))

                with tc.tile_pool(name="ps_fh", bufs=1, space="PSUM") as fh_ps_pool:
                    # [128, 1024] split layout (2 PSUM banks instead of 4):
                    # chunks 0,1 live on partitions 0..63, chunks 2,3 on
                    # 64..127. Frees banks so einsum3's PSUM pool can coexist
                    # with the transpose pools (einsum3 interleaving below).
                    ps_fh = fh_ps_pool.tile([P, 2 * CHUNK], f32, tag="fh")

                    def fh_ps(c):
                        return ps_fh[(c // 2) * R:(c // 2 + 1) * R,
                                     (c % 2) * CHUNK:(c % 2 + 1) * CHUNK]

                    def e1(g):
                        # einsum1: accumulate partial F_h chunks from group g
                        raw, off = raws[g]
                        for q in range(4 if lvl >= 1 else 0):
                            t = 4 * g + q
                            base = off + q * H
                            for c in range(NCH):
                                nc.tensor.matmul(
                                    fh_ps(c),
                                    wst_t[:, t * R:(t + 1) * R],
                                    raw[:, base + c * CHUNK:base + (c + 1) * CHUNK],
                                    start=(t == 0),
                                    stop=(t == n_t - 1),
                                )

                    def tr_e2(g, tr_ps, fs_ps, unit_fn=None):
                        # einsum2 for own group g (s-tiles 4g..4g+3).
                        # einsum2's matmul for h-tile k is emitted after the
                        # transposes for k+1 (software pipelining), so the PE
                        # never stalls on the PSUM->SBUF copy of k. unit_fn
                        # (if given) interleaves one einsum3 unit per k.
                        raw, off = raws[g]
                        ps_fs_g = None
                        if lvl >= 3:
                            ps_fs_g = fs_ps.tile([R, CHUNK], f32, tag="fs")
                        hsTs = {}
                        # h-tiles processed in PAIRS: 8 transposes share one
                        # PSUM tile and one (wider) PSUM->SBUF copy, halving
                        # copy-instruction overhead on DVE/Act
                        for kk in range(T // 2 + 1):  # k-pairs (+1 drain)
                            if kk < T // 2:
                                ps_tr = tr_ps.tile([P, 8 * P], f16, tag="tr")
                                for k2 in range(2):
                                    k = 2 * kk + k2
                                    for q in range(4):  # s-tiles 4g+q
                                        qb = off + q * H
                                        src = raw[:, qb + k * P:qb + (k + 1) * P]
                                        nc.tensor.matmul(
                                            ps_tr[:, (4 * k2 + q) * P:(4 * k2 + q + 1) * P],
                                            src,
                                            id_t[:],
                                            is_transpose=True,
                                            start=(k2 == 0 and q == 0),
                                            stop=(k2 == 1 and q == 3),
                                        )
                                hsT = hpool.tile([P, 8 * P], f16, tag="hsT")
                                if kk % 2 == 1:
                                    nc.scalar.copy(hsT[:], ps_tr[:])
                                else:
                                    nc.vector.tensor_copy(hsT[:], ps_tr[:])
                                hsTs[kk] = hsT
                            if lvl >= 3 and kk > 0:
                                hsT_p = hsTs.pop(kk - 1)
                                for k2 in range(2):
                                    k = 2 * (kk - 1) + k2
                                    nc.tensor.matmul(
                                        ps_fs_g[:],
                                        wht_t[:, k * R:(k + 1) * R],
                                        hsT_p[:, 4 * k2 * P:4 * (k2 + 1) * P],
                                        start=(k == 0),
                                        stop=(k == T - 1),
                                    )
                            if unit_fn is not None and kk >= 1:
                                unit_fn()
                        if lvl >= 3:
                            # F_s.T chunk -> SBUF, duplicated to both halves
                            nc.vector.tensor_copy(fst_g[g][0:R, :], ps_fs_g[:])
                            nc.scalar.copy(fst_g[g][R:2 * R, :], ps_fs_g[:])

                    def fh_copies():
                        # F_h -> SBUF, duplicated to both partition halves.
                        # Emitted before tr_e2(1): the copies run on DVE/Act
                        # while the PE does group 1's transposes, so einsum3
                        # starts with no PE idle (keeps the PE p-state high).
                        if lvl < 4:
                            return
                        # 4 copies of [64, 1024]; ph = psum partition half
                        # (fh cols ph*1024..): DVE serves dup-half 0, Act
                        # dup-half 1, lowest columns first so einsum3's
                        # earliest units unblock first.
                        for ph in range(2):
                            src = ps_fh[ph * R:(ph + 1) * R, :]
                            sl = slice(ph * 2 * CHUNK, (ph + 1) * 2 * CHUNK)
                            nc.vector.tensor_copy(fh_dup[0:R, sl], src)
                            nc.scalar.copy(fh_dup[R:2 * R, sl], src)

                    def fh_allreduce():
                        # partial F_h -> fp16 SBUF -> DRAM bounce -> pairwise
                        # AllReduce -> back into both halves of fh_dup. The
                        # whole chain overlaps the PE's transpose phase.
                        if lvl < 4:
                            return
                        fhp = fpool.tile([R, H], f16, tag="fhp")
                        for ph in range(2):
                            src = ps_fh[ph * R:(ph + 1) * R, :]
                            sl = slice(ph * 2 * CHUNK, (ph + 1) * 2 * CHUNK)
                            if ph == 0:
                                nc.vector.tensor_copy(fhp[:, sl], src)
                            else:
                                nc.scalar.copy(fhp[:, sl], src)
                        in_b = ccpool.tile([R, H], f16, tag="cc_in")
                        out_b = ccpool.tile([R, H], f16, tag="cc_out")
                        nc.scalar.dma_start(in_b[:], fhp[:])
                        nc.gpsimd.collective_compute(
                            "AllReduce",
                            mybir.AluOpType.add,
                            replica_groups=CC_GROUPS,
                            ins=[in_b.opt()],
                            outs=[out_b.opt()],
                        )
                        nc.gpsimd.dma_start(fh_dup[0:R, :], out_b[:])
                        nc.gpsimd.dma_start(fh_dup[R:2 * R, :], out_b[:])

                    # einsum3 (K=64, row-packed pairs) + output stores,
                    # emitted one [128, 512] unit at a time. ip0/ip1's 16
                    # units interleave into tr_e2(1)'s PE stream (they only
                    # need fst_g[0] + fh_dup, both ready by then), spreading
                    # the PSUM->SBUF cast copies across the transpose phase.
                    e3_state = {"n": 0, "outsb": None}

                    def e3_unit(po_pool):
                        if lvl < 4:
                            return
                        # emits a PAIR of adjacent-c units sharing one PSUM
                        # tile and one wide [128, 1024] cast copy
                        n = e3_state["n"]
                        if n >= 32:
                            return
                        e3_state["n"] = n + 2
                        ip, u = n // 8, n % 8
                        hf, c0 = u // 4, u % 4
                        i = 2 * ip + hf
                        base = R * hf
                        if n % 16 == 0:
                            # one SBUF staging tile (and one 16KB/partition
                            # store) per PAIR of ips
                            e3_state["outsb"] = opool.tile(
                                [P, 4 * H], f16, tag="outsb", name="outsb"
                            )
                        outsb = e3_state["outsb"]
                        ps_o = po_pool.tile([P, 2 * CHUNK], f32, tag="o")
                        for cc in range(2):
                            nc.tensor.matmul(
                                ps_o[:, cc * CHUNK:(cc + 1) * CHUNK],
                                fst_g[i // 4][base:base + R, (i % 4) * P:(i % 4 + 1) * P],
                                fh_dup[base:base + R, (c0 + cc) * CHUNK:(c0 + cc + 1) * CHUNK],
                                start=True,
                                stop=True,
                            )
                        col = (ip % 2) * 2 * H + hf * H + c0 * CHUNK
                        dst = outsb[:, col:col + 2 * CHUNK]
                        if (n // 2) % 2 == 0:
                            nc.vector.tensor_copy(dst, ps_o[:])
                        else:
                            nc.scalar.copy(dst, ps_o[:])
                        if n % 16 == 14 and not skip_stores:
                            nc.gpsimd.dma_start(out[n // 16], outsb[:])

                    with (
                        tc.tile_pool(name="ps_tr", bufs=2, space="PSUM") as tr_ps,
                        tc.tile_pool(name="ps_fs", bufs=2, space="PSUM") as fs_ps,
                    ):
                        if COLLECTIVE:
                            e1(0)
                            e1(1)
                            fh_allreduce()
                            if lvl >= 2:
                                tr_e2(0, tr_ps, fs_ps)
                                tr_e2(1, tr_ps, fs_ps)
                        else:
                            # own half first: e1(0) + both transpose/e2 groups,
                            # so raw0/raw1 (and their load slots) free early and
                            # the only late dependency is e3's wait on the full
                            # peer half (unavoidable: F_h sums all of s)
                            e1(0)
                            if lvl >= 2:
                                tr_e2(0, tr_ps, fs_ps)
                            e1(1)
                            if lvl >= 2:
                                tr_e2(1, tr_ps, fs_ps)
                            e1(2)
                            e1(3)
                            fh_copies()

                # einsum3 with a deep PSUM pool so the PE runs far ahead of
                # the cast copies
                with tc.tile_pool(name="ps_o", bufs=3, space="PSUM") as po_b:
                    if lvl < 4:
                        if not skip_stores:
                            for sp in range(NG // 2):
                                nc.gpsimd.dma_start(out[sp], fake_out[:])
                    else:
                        while e3_state["n"] < 32:
                            e3_unit(po_b)

            if unroll:
                for _ in range(reps):
                    body()
            elif reps == 1:
                body()
            else:
                nbody = 8 if reps % 8 == 0 else (4 if reps % 4 == 0 else 2)
                assert reps % nbody == 0, "reps must be even (2 bodies per For_i)"
                with tc.For_i(0, reps // nbody, 1):
                    for _ in range(nbody):
                        body()

    nc.compile()
    return nc


def _tile_weight(w_t: np.ndarray) -> np.ndarray:
    """[2048, 64] -> [128, 16*64] stack where tile t = cols [64t:64t+64]."""
    return np.ascontiguousarray(
        w_t.reshape(T, P, R).transpose(1, 0, 2).reshape(P, T * R)
    ).astype(np.float16)


def _pack_hs(hs_c: np.ndarray) -> np.ndarray:
    """[rows, 2048] -> [rows/512, 128, 4*H]: group g = s-tiles 4g..4g+3, row p
    of group g holds tiles' rows s = g*512 + q*128 + p concatenated over q."""
    ng = hs_c.shape[0] // 512
    return np.ascontiguousarray(
        hs_c.reshape(ng, 4, P, H).transpose(0, 2, 1, 3).reshape(ng, P, 4 * H)
    )


def _unpack_out(o: np.ndarray) -> np.ndarray:
    """[2, 128, 4*H] -> [1024, 2048]: s = (2*sp + ipw)*256 + w*128 + p."""
    return np.ascontiguousarray(
        o.reshape(2, P, 2, 2, H).transpose(0, 2, 3, 1, 4).reshape(S // 2, H)
    )


def _tile_weight_half(w_t: np.ndarray) -> np.ndarray:
    """[1024, 64] -> [128, 8*64] stack where tile t = cols [64t:64t+64]."""
    return np.ascontiguousarray(
        w_t.reshape(T // 2, P, R).transpose(1, 0, 2).reshape(P, T // 2 * R)
    ).astype(np.float16)


def build_in_maps(hs_all: np.ndarray, w_seq: np.ndarray, w_hid: np.ndarray):
    ident = np.eye(P, dtype=np.float16)
    wht_tiled = _tile_weight(np.ascontiguousarray(w_hid.T))
    hs_f16 = hs_all.astype(np.float16)
    if COLLECTIVE:
        wst_halves = {
            j: _tile_weight_half(
                np.ascontiguousarray(w_seq.T[j * (S // 2):(j + 1) * (S // 2)])
            )
            for j in range(2)
        }
    else:
        wst_halves = {
            j: _tile_weight(np.roll(w_seq.T, -(S // 2) * j, axis=0))
            for j in range(2)
        }
    in_maps = []
    for c in range(N_CORES):
        b, j = c // 2, c % 2
        hsb = hs_f16[b]
        if COLLECTIVE:
            hs_c = hsb[j * (S // 2):(j + 1) * (S // 2)]
        else:
            hs_c = hsb if j == 0 else np.roll(hsb, -(S // 2), axis=0)
        in_maps.append(
            {"hsg": _pack_hs(hs_c), "wst": wst_halves[j], "wht": wht_tiled,
             "ident": ident}
        )
    return in_maps


_NC_CACHE: dict = {}


def kernel(**inputs) -> np.ndarray:
    hs_all = np.asarray(inputs["hidden_states"], dtype=np.float32)
    w_seq = np.asarray(inputs["W_seq"], dtype=np.float32)
    w_hid = np.asarray(inputs["W_hid"], dtype=np.float32)

    if "nc" not in _NC_CACHE:
        _NC_CACHE["nc"] = build_nc(1)
    nc = _NC_CACHE["nc"]

    in_maps = build_in_maps(hs_all, w_seq, w_hid)
    res = run_bass_kernel_spmd(nc, in_maps, core_ids=list(range(N_CORES)))

    out_full = np.empty((B, S, H), dtype=np.float32)
    for c in range(N_CORES):
        b, j = c // 2, c % 2
        out_full[b, j * (S // 2):(j + 1) * (S // 2), :] = _unpack_out(
            res.results[c]["out"]
        )
    return out_full



# revision 43
# speedup vs baseline: 1.1848x; 1.1848x over previous
"""Trainium2 Bass kernel for nn_LlamaApproximatedAttention.

Math (per batch b, with hs = hidden_states[b] [S, H]):
    F_h = W_seq @ hs            # [R, H]   (contract s)
    F_s = hs @ W_hid.T          # [S, R]   (contract h)
    out = F_s @ F_h             # [S, H]   (contract r)

Sharding: 8 cores = (batch b = c//2, seq-half j = c%2). Each core receives the
full hs[b] with its own half's rows first (host-side roll), computes F_h fully,
and F_s / out only for its own half. Pure SPMD, no collectives (a pairwise
AllReduce of F_h would halve load DMA + e1 but costs ~20us latency).

Empirical cost model this schedule is built around (HW-measured micro):
  - matmul moving operand streams ~1 col/cycle at 2.4 GHz when both operands
    span 128 partitions; HALF rate when they span only 64 (rank-64 contraction)
  - 64-col stationaries (e1/e2) with rotating PSUM quadrants stream ~2x
  - batched PE transposes with the identity kept loaded: ~58 ns per 128x128
  - interleaving different stationaries (identity/weights) costs ~0.5us/swap
  - PSUM->SBUF copies (DVE/Act only) cost ~0.5-1us each incl. the pipe drain

Key trick: einsum2 keeps its two half-contractions (h-tiles 0..7 / 8..15) as
SEPARATE partials stacked on partitions [F_sA; F_sB] [128, own-S], and einsum3
contracts all 128 partitions against [F_h; F_h] (fh duplicated into both
halves): F_sA.T@F_h + F_sB.T@F_h == F_s.T'@F_h == own out rows. That turns the
rank-64 einsum3 into a full-width K=128 matmul (2x PE throughput), for free.

DMA: hs pre-packed on host into 2 loads of [128, 8*H] bf16 (own half first);
loads on the SP HWDGE queue, stores (4 x 1 MiB) on the Pool SWDGE queue.
Measured floor for the 12.6 MiB/body of HBM traffic is ~38.5us; the whole
compute stream (~28us) hides under it.

grid_chw is unused by the math (it enumerates the full (s, h) grid).
"""

import numpy as np

import concourse.bass as bass  # noqa: F401  (engine namespaces hang off nc)
import concourse.mybir as mybir
import concourse.tile as tile
from concourse import bacc
from concourse.bass_utils import run_bass_kernel_spmd

B, S, H, R = 4, 2048, 2048, 64
N_CORES = 8
P = 128
T = S // P            # 16 s-tiles (also 16 h-tiles)
OWN_T = T // 2        # 8 own s-tiles per core
CHUNK = 512
NCH = H // CHUNK      # 4 h-chunks
NG = 4                # s-tile groups of 4
GROUPS_PER_LOAD = 2   # 2 -> 2 load DMAs of 4 MiB

f16 = mybir.dt.bfloat16
f32 = mybir.dt.float32
np_f16 = mybir.dt.np(f16)


def build_nc(reps: int = 1, mode: str = "full", unroll: bool = False):
    """Build + bacc-compile the SPMD kernel. reps>1 wraps copies of the body
    in a hardware For loop, so tile-pool rotation double-buffers across
    iterations. Output is idempotent.

    mode (timing diagnostics, progressively enables phases):
      "dma"  : input loads + output stores only
      "e1"   : + einsum1 matmuls
      "tr"   : + PE transposes
      "e2"   : + einsum2 matmuls/copies
      "full" : + einsum3 (the real kernel)
    mode may carry suffixes "-noload" (tiny input DMAs only; compute on
    garbage SBUF) and/or "-nostore" (skip output DMAs) for isolation.
    """
    skip_loads = "-noload" in mode
    skip_stores = "-nostore" in mode
    skip_e3_copies = "-nocopy3" in mode
    const_e3 = "-conste3" in mode
    only_e3 = "-only3" in mode
    mode = mode.split("-")[0]
    lvl = {"dma": 0, "e1": 1, "tr": 2, "e2": 3, "full": 4}[mode]
    nc = bacc.Bacc(
        "TRN2",
        target_bir_lowering=False,
        debug=False,
        enable_asserts=True,
        num_devices=N_CORES,
    )

    # own half: one 4 MiB load [128, 8*H]; peer half: four 1 MiB chunk loads
    # [128, 2*H] streamed through a small rotating buffer (saves 32KB SBUF,
    # which buys a dedicated staging tile per output store)
    hso = nc.dram_tensor("hso", [P, 8 * H], f16, kind="ExternalInput").ap()
    hsp = nc.dram_tensor("hsp", [4, P, 2 * H], f16, kind="ExternalInput").ap()
    wst = nc.dram_tensor("wst", [P, T * R], f16, kind="ExternalInput").ap()
    wht = nc.dram_tensor("wht", [P, T * R], f16, kind="ExternalInput").ap()
    ident = nc.dram_tensor("ident", [P, P], f16, kind="ExternalInput").ap()
    # store ii holds own s-tile 4*(ii%2) + ii//2: [128, H] bf16 (0.5 MiB)
    out = nc.dram_tensor("out", [OWN_T, P, H], f16,
                         kind="ExternalOutput").ap()

    with tile.TileContext(nc) as tc:
        with (
            tc.tile_pool(name="const", bufs=1) as cpool,
            tc.tile_pool(name="raw", bufs=2) as rpool,
            tc.tile_pool(name="hsT", bufs=1) as hpool,
            tc.tile_pool(name="fact", bufs=1) as fpool,
            tc.tile_pool(name="outsb", bufs=1) as opool,
        ):
            # constants: loaded once, outside the timing loop
            wst_t = cpool.tile([P, T * R], f16, tag="wst")
            wht_t = cpool.tile([P, T * R], f16, tag="wht")
            id_t = cpool.tile([P, P], f16, tag="id")
            nc.gpsimd.dma_start(wst_t[:], wst)
            nc.gpsimd.dma_start(wht_t[:], wht)
            nc.gpsimd.dma_start(id_t[:], ident)
            fake_out = None
            if lvl < 4:
                fake_out = cpool.tile([P, H], f16, tag="fake_out")
                nc.gpsimd.memset(fake_out[:], 0.0)
            cfsta = cfstb = cfh = None
            if const_e3:
                cfsta = cpool.tile([P, 4 * P], f16, tag="cfsta")
                cfstb = cpool.tile([P, 4 * P], f16, tag="cfstb")
                cfh = cpool.tile([P, H], f16, tag="cfh")
                nc.vector.memset(cfsta[:], 0.5)
                nc.vector.memset(cfstb[:], 0.5)
                nc.vector.memset(cfh[:], 0.5)

            # alternate PSUM->SBUF cast copies between DVE and Act
            cp_state = {"n": 0}

            def cast_copy(dst, src):
                if cp_state["n"] % 2 == 0:
                    nc.vector.tensor_copy(dst, src)
                else:
                    nc.scalar.copy(dst, src)
                cp_state["n"] += 1

            def body(_=None):
                # [F_sA; F_sB] stacked partials of F_s.T (see module
                # docstring), split into two SBUF tiles (own s-tiles 0..3 /
                # 4..7) so einsum3 can alternate its stationary source tile
                # (lets the LDW of step ii+1 pull ahead of step ii's matmuls)
                fst2a = fpool.tile([P, 4 * P], f16, tag="fsta")    # [128,512]
                fst2b = fpool.tile([P, 4 * P], f16, tag="fstb")
                fh_dup = fpool.tile([P, H], f16, tag="fh")         # [128,2048]

                own = rpool.tile([P, 8 * H], f16, tag="own")
                if not skip_loads:
                    nc.sync.dma_start(own[:], hso)
                else:
                    nc.sync.dma_start(own[:, 0:16], hso[:, 0:16])
                peers = []
                for pc in range(4):
                    pt = rpool.tile([P, 2 * H], f16, tag=f"peer{pc % 2}",
                                    name=f"peer{pc}")
                    if not skip_loads:
                        nc.sync.dma_start(pt[:], hsp[pc])
                    else:
                        nc.sync.dma_start(pt[:, 0:16], hsp[pc, :, 0:16])
                    peers.append(pt)

                def s_tile_ap(t, k):
                    """[128, 128] slice of s-tile t, h-tile k."""
                    if t < OWN_T:
                        return own[:, t * H + k * P:t * H + (k + 1) * P]
                    pt = peers[(t - OWN_T) // 2]
                    off = ((t - OWN_T) % 2) * H
                    return pt[:, off + k * P:off + (k + 1) * P]

                with (
                    tc.tile_pool(name="ps_fh", bufs=1, space="PSUM") as fh_psp,
                ):
                    # F_h accumulator, split layout: chunks 0,1 on partitions
                    # 0..63, chunks 2,3 on 64..127 (2 banks)
                    ps_fh = fh_psp.tile([P, 2 * CHUNK], f32, tag="fh")

                    def fh_ps(c):
                        return ps_fh[(c // 2) * R:(c // 2 + 1) * R,
                                     (c % 2) * CHUNK:(c % 2 + 1) * CHUNK]

                    def e1(g):
                        # einsum1: accumulate partial F_h from s-group g
                        for q in range(4 if lvl >= 1 else 0):
                            t = 4 * g + q
                            if t < OWN_T:
                                src, base = own, t * H
                            else:
                                src = peers[(t - OWN_T) // 2]
                                base = ((t - OWN_T) % 2) * H
                            for c in range(NCH):
                                nc.tensor.matmul(
                                    fh_ps(c),
                                    wst_t[:, t * R:(t + 1) * R],
                                    src[:, base + c * CHUNK:base + (c + 1) * CHUNK],
                                    start=(t == 0),
                                    stop=(t == T - 1),
                                )

                    hsTs = {}

                    def tr_all(trp):
                        # transpose own 8 s-tiles for ALL 16 h-tiles in one
                        # batch (identity stays loaded throughout); copies
                        # drain to SBUF on DVE/Act behind the PE
                        if lvl < 2:
                            return
                        for k in range(T):
                            ps_tr = trp.tile([P, OWN_T * P], f16, tag="tr")
                            for q in range(OWN_T):
                                nc.tensor.matmul(
                                    ps_tr[:, q * P:(q + 1) * P],
                                    s_tile_ap(q, k),
                                    id_t[:],
                                    is_transpose=True,
                                    start=(q == 0),
                                    stop=(q == OWN_T - 1),
                                )
                            hsT = hpool.tile([P, OWN_T * P], f16,
                                             tag=f"hsT{k}")
                            cast_copy(hsT[:], ps_tr[:])
                            hsTs[k] = hsT

                    def e2_all(fs_psp):
                        # einsum2, both h-halves interleaved: partial A
                        # (h-tiles 0..7) accumulates on partitions 0:64 /
                        # cols 0:1024, partial B (8..15) on partitions 64:128
                        # / cols 1024:2048 -> consecutive matmuls rotate over
                        # 4 PSUM banks (same-bank accumulate back-to-back is
                        # ~2x slower)
                        if lvl < 3:
                            return None
                        ps_fs = fs_psp.tile([P, 4 * CHUNK], f32, tag="fs",
                                            name="ps_fs")
                        for j in range(8):
                            for cc in range(2):
                                for hp in range(2):
                                    k = hp * 8 + j
                                    nc.tensor.matmul(
                                        ps_fs[hp * R:(hp + 1) * R,
                                              hp * 2 * CHUNK + cc * CHUNK:
                                              hp * 2 * CHUNK + (cc + 1) * CHUNK],
                                        wht_t[:, k * R:(k + 1) * R],
                                        hsTs[k][:, cc * CHUNK:(cc + 1) * CHUNK],
                                        start=(j == 0),
                                        stop=(j == 7),
                                    )
                        # F_s.T partials -> SBUF: a = own s-tiles 0..3,
                        # b = 4..7 (contiguous 512-col slices of each half)
                        cast_copy(fst2a[0:R, :], ps_fs[0:R, 0:4 * P])
                        cast_copy(fst2b[0:R, :], ps_fs[0:R, 4 * P:8 * P])
                        cast_copy(fst2a[R:P, :],
                                  ps_fs[R:P, 2 * CHUNK:2 * CHUNK + 4 * P])
                        cast_copy(fst2b[R:P, :],
                                  ps_fs[R:P, 2 * CHUNK + 4 * P:4 * CHUNK])
                        return ps_fs

                    def fh_copies():
                        # F_h -> SBUF, duplicated to both partition halves
                        if lvl < 4:
                            return
                        for ph in range(2):
                            src = ps_fh[ph * R:(ph + 1) * R, :]
                            sl = slice(ph * 2 * CHUNK, (ph + 1) * 2 * CHUNK)
                            nc.vector.tensor_copy(fh_dup[0:R, sl], src)
                            nc.scalar.copy(fh_dup[R:2 * R, sl], src)

                    # all of e1 first: fh_copies (which gate einsum3) are
                    # emitted before the transpose phase so they drain on
                    # DVE/Act while the PE transposes; einsum3's entry then
                    # only waits on the fst copies
                    # Phase order: TR -> e1(own) -> E2 -> e1(peer) -> E3.
                    # The copy-gated TR phase runs first; the dense e1/e2
                    # stream then keeps the PE clock warm (HAM) before
                    # einsum3. e1(own) right after TR releases the raw0 load
                    # slot by ~55% of the PE stream (lets the next-next
                    # body's first load start early); the peer-half load is
                    # only needed at ~80%.
                    if not only_e3:
                        with tc.tile_pool(name="ps_tr", bufs=4,
                                          space="PSUM") as trp:
                            tr_all(trp)
                        e1(0)
                        e1(1)
                        e1(2)
                        e1(3)
                        fh_copies()  # drain on DVE/Act during e2's matmuls
                        with tc.tile_pool(name="ps_fs", bufs=1,
                                          space="PSUM") as fs_psp:
                            e2_all(fs_psp)

                # einsum3, K=128: out s-tile i = fst2[:, i].T @ [F_h; F_h]
                with tc.tile_pool(name="ps_o", bufs=4, space="PSUM") as po:
                    if lvl < 4:
                        if not skip_stores:
                            for sp in range(OWN_T):
                                nc.gpsimd.dma_start(out[sp], fake_out[:])
                        return
                    # visit own s-tiles in order 0,4,1,5,2,6,3,7 so the
                    # stationary alternates between fst2a and fst2b (enables
                    # LDW pull-ahead); each s-tile gets its own staging tile
                    # and store, so copies never wait on store completions
                    for ii in range(OWN_T):
                        tile_i = (ii // 2) * P
                        if const_e3:
                            stat = cfsta if ii % 2 == 0 else cfstb
                        else:
                            stat = fst2a if ii % 2 == 0 else fst2b
                        outsb = opool.tile([P, H], f16, tag=f"outsb{ii}",
                                           name=f"outsb{ii}")
                        for c0 in range(2):
                            ps_o = po.tile([P, 2 * CHUNK], f32, tag="o")
                            for cc in range(2):
                                nc.tensor.matmul(
                                    ps_o[:, cc * CHUNK:(cc + 1) * CHUNK],
                                    stat[:, tile_i:tile_i + P],
                                    (cfh if const_e3 else fh_dup)[
                                        :, (2 * c0 + cc) * CHUNK:
                                        (2 * c0 + cc + 1) * CHUNK],
                                    start=True,
                                    stop=True,
                                )
                            if not skip_e3_copies:
                                cast_copy(
                                    outsb[:, c0 * 2 * CHUNK:
                                          (c0 + 1) * 2 * CHUNK],
                                    ps_o[:],
                                )
                        if not skip_stores and not skip_e3_copies:
                            nc.gpsimd.dma_start(out[ii], outsb[:])

            if unroll:
                for _ in range(reps):
                    body()
            elif reps == 1:
                body()
            else:
                nbody = 8 if reps % 8 == 0 else (4 if reps % 4 == 0 else 2)
                assert reps % nbody == 0
                with tc.For_i(0, reps // nbody, 1):
                    for _ in range(nbody):
                        body()

    nc.compile()
    return nc


def _tile_weight(w_t: np.ndarray) -> np.ndarray:
    """[2048, 64] -> [128, 16*64] stack where tile t = cols [64t:64t+64]."""
    return np.ascontiguousarray(
        w_t.reshape(T, P, R).transpose(1, 0, 2).reshape(P, T * R)
    ).astype(np_f16)


def _pack_half(hs_h: np.ndarray) -> np.ndarray:
    """[1024, 2048] -> [128, 8*H]: row p holds s-tile rows 128*t + p
    concatenated over the half's 8 s-tiles t."""
    return np.ascontiguousarray(
        hs_h.reshape(8, P, H).transpose(1, 0, 2).reshape(P, 8 * H)
    )


def _unpack_out(o: np.ndarray) -> np.ndarray:
    """[8, 128, H] -> [1024, 2048]: store ii holds s-tile 4*(ii%2) + ii//2,
    so s-tile t comes from store index 2*(t%4) + (t>=4)."""
    return np.ascontiguousarray(
        o[[0, 2, 4, 6, 1, 3, 5, 7]].reshape(S // 2, H)
    )


def build_in_maps(hs_all: np.ndarray, w_seq: np.ndarray, w_hid: np.ndarray):
    ident = np.eye(P, dtype=np_f16)
    wht_tiled = _tile_weight(np.ascontiguousarray(w_hid.T))
    hs_f16 = hs_all.astype(np_f16)
    wst_halves = {
        j: _tile_weight(np.roll(w_seq.T, -(S // 2) * j, axis=0))
        for j in range(2)
    }
    in_maps = []
    for c in range(N_CORES):
        b, j = c // 2, c % 2
        hsb = hs_f16[b]
        own_h = hsb[j * (S // 2):(j + 1) * (S // 2)]
        peer_h = hsb[(1 - j) * (S // 2):(2 - j) * (S // 2)]
        in_maps.append(
            {"hso": _pack_half(own_h),
             "hsp": _pack_half(peer_h).reshape(P, 4, 2 * H)
                    .transpose(1, 0, 2).copy(),
             "wst": wst_halves[j], "wht": wht_tiled, "ident": ident}
        )
    return in_maps


_NC_CACHE: dict = {}


def kernel(**inputs) -> np.ndarray:
    hs_all = np.asarray(inputs["hidden_states"], dtype=np.float32)
    w_seq = np.asarray(inputs["W_seq"], dtype=np.float32)
    w_hid = np.asarray(inputs["W_hid"], dtype=np.float32)

    if "nc" not in _NC_CACHE:
        _NC_CACHE["nc"] = build_nc(1)
    nc = _NC_CACHE["nc"]

    in_maps = build_in_maps(hs_all, w_seq, w_hid)
    res = run_bass_kernel_spmd(nc, in_maps, core_ids=list(range(N_CORES)))

    out_full = np.empty((B, S, H), dtype=np.float32)
    for c in range(N_CORES):
        b, j = c // 2, c % 2
        out_full[b, j * (S // 2):(j + 1) * (S // 2), :] = _unpack_out(
            res.results[c]["out"]
        )
    return out_full
